# revision 4
# baseline (speedup 1.0000x reference)
"""CAM (channel attention) module kernel for Trainium2, 8-core data-parallel.

Computes, per batch b (one batch per NeuronCore):
    q = x[b].reshape(C, N)                  # C=512, N=4096
    E = q @ q.T                             # [C, C], symmetric
    att = softmax(rowmax(E) - E, axis=-1)   # == exp(rowmin(E)-E)/rowsum
    out = gamma * (att @ q) + x[b]

v3 design (trace-driven rework of v2; all matmuls bf16, fp32 E in PSUM,
exact fp32 +x via scalar_tensor_tensor so gamma=0 gives out == x exactly):
  - loads: 13 ramped slabs [p, c, n] (x viewed as (c p) n -> p c n) split
    across the two HWDGE rings (sync/scalar), issued first so the rings
    drain in FIFO order at full HBM rate; small slabs first (fast pipe
    start) and last (short tail chain).
  - PE warmup: a few junk matmuls on a memset scratch tile right at t~6us
    so the HAM clock-gate is released before real work arrives.
  - ALL q transposes on PE via identity (bf16, 56ns per 128x128) into
    PSUM staging, gathered to qT by DVE/ACT copies. No DMA-xbar for qT:
    in v2 the xbar transposes crawled behind the load stream and delayed
    tail energy by ~14us.
  - casts fp32->bf16 and PSUM gathers alternate DVE/ACT per c-chunk.
  - energy k-outer i-inner per slab (j>=i blocks only); last slab runs
    i-outer with stop so E[0] completes first; j<i blocks mirrored from
    E[j] (ACT copy out + PE fp32 transpose back).
  - softmax: rowmin (DVE) -> exp(mn-E) on ACT (bf16 att + accum_out row
    sum); rg = gamma/s per partition.
  - attT via PE transposes + ACT copy (v2 used xbar: +2.5us latency on
    the critical path at the energy->out transition).
  - out: per 512-col chunk: 4 bf16 matmuls (lhsT=attT) into a 1-bank
    PSUM tile, then DVE scalar_tensor_tensor out = psum*rg + x (exact
    fp32 x), store per chunk alternating sync/scalar rings (no SWDGE).
"""

import sys

import numpy as np

for _p in ("/opt/trn_rl_repo",):
    if _p not in sys.path:
        sys.path.insert(0, _p)

B, C, H, W = 8, 512, 64, 64
N = H * W  # 4096
P = 128
CT = C // P  # 4 channel tiles
KT = N // P  # 32 spatial tiles
FD = 512  # matmul free-dim / PSUM bank width (fp32)

# ramped load slabs (columns); sum = 4096
SLAB_COLS = [128, 128, 128, 128, 256, 256, 512, 512, 512, 512, 512, 256, 256]
assert sum(SLAB_COLS) == N

_CACHE = {}


def _build_bass():
    import concourse.mybir as mybir
    import concourse.tile as tile
    from concourse import bacc
    from concourse.masks import make_identity

    fp32 = mybir.dt.float32
    bf16 = mybir.dt.bfloat16
    AX = mybir.AxisListType.X
    ALU = mybir.AluOpType
    ACT_EXP = mybir.ActivationFunctionType.Exp

    nc = bacc.Bacc(None, target_bir_lowering=False, debug=False)
    x_d = nc.dram_tensor("x", [C, N], fp32, kind="ExternalInput")
    g_d = nc.dram_tensor("gamma", [1], fp32, kind="ExternalInput")
    o_d = nc.dram_tensor("out", [C, N], fp32, kind="ExternalOutput")

    with tile.TileContext(nc) as tc:
        with (
            tc.tile_pool(name="persist", bufs=1) as persist,
            tc.tile_pool(name="stats", bufs=4) as stats,
            tc.tile_pool(name="outp", bufs=4) as outp,
            tc.tile_pool(name="eps", bufs=4, space="PSUM") as eps,
            tc.tile_pool(name="ps", bufs=4, space="PSUM") as ps,
        ):
            gam = persist.tile([P, 1], fp32)
            ident = persist.tile([P, P], bf16)
            ident32 = persist.tile([P, P], fp32)
            scratch = persist.tile([P, FD], bf16)
            q = persist.tile([P, CT, N], fp32)
            q_bf = persist.tile([P, CT, N], bf16)
            # qT[p, k, c, v] = q[c*128+v, k*128+p]; energy rhs for block k
            # is the contiguous [128, 512] slab qT[:, k, :, :]
            qT = persist.tile([P, KT, CT, P], bf16)
            att = persist.tile([P, CT, C], bf16)
            attT = persist.tile([P, CT, CT, P], bf16)

            # ---- slab loads, issued first so the rings drain FIFO ----
            xv = x_d[:, :].rearrange("(c p) n -> p c n", p=P)
            slabs = []
            c0 = 0
            for s, w in enumerate(SLAB_COLS):
                sl = slice(c0, c0 + w)
                slabs.append((sl, c0 // P, w // P))  # (cols, k0, kb)
                c0 += w
                ring = nc.sync if s % 2 == 0 else nc.scalar
                ring.dma_start(out=q[:, :, sl], in_=xv[:, :, sl])

            # ---- PE warmup: junk matmuls to release the HAM clock gate ----
            nc.gpsimd.memset(scratch, 0.0)
            for _ in range(7):
                wp = ps.tile([P, FD], fp32, name="wp", tag="ps")
                nc.tensor.matmul(
                    wp, lhsT=scratch[:, :P], rhs=scratch, start=True, stop=True
                )
            make_identity(nc, ident)
            make_identity(nc, ident32)
            nc.gpsimd.dma_start(out=gam, in_=g_d[:].to_broadcast((P, 1)))

            Es = [
                eps.tile([P, C], fp32, name=f"E{i}", tag=f"E{i}", bufs=1)
                for i in range(CT)
            ]

            def cast(sl, c):
                eng = nc.vector if c % 2 == 0 else nc.scalar
                if c % 2 == 0:
                    eng.tensor_copy(out=q_bf[:, c, sl], in_=q[:, c, sl])
                else:
                    eng.copy(out=q_bf[:, c, sl], in_=q[:, c, sl])

            def transpose_gather(sl, k0, kb, c):
                tp = ps.tile([P, 2 * FD], bf16, name="tp", tag="ps")
                for kk in range(kb):
                    nc.tensor.transpose(
                        tp[:, kk * P : (kk + 1) * P],
                        q_bf[:, c, sl.start + kk * P : sl.start + (kk + 1) * P],
                        ident,
                    )
                src = tp[:, 0 : kb * P].rearrange("p (k v) -> p k v", v=P)
                dst = qT[:, k0 : k0 + kb, c, :]
                if c % 2 == 0:
                    nc.vector.tensor_copy(out=dst, in_=src)
                else:
                    nc.scalar.copy(out=dst, in_=src)

            def energy_k(k, i, stop=False):
                nc.tensor.matmul(
                    Es[i][:, i * P :],
                    lhsT=qT[:, k, i, :],
                    rhs=qT[:, k, i:, :],
                    start=(k == 0),
                    stop=stop,
                )

            # ---- slab pipeline (all but last slab: k-outer, i-inner) ----
            for s, (sl, k0, kb) in enumerate(slabs):
                for c in range(CT):
                    cast(sl, c)
                for c in range(CT):
                    transpose_gather(sl, k0, kb, c)
                if s < len(slabs) - 1:
                    for kk in range(kb):
                        for i in range(CT):
                            energy_k(k0 + kk, i)

            # ---- transition: last-slab energy i-outer + mirrors ----
            lsl, lk0, lkb = slabs[-1]
            mn = [stats.tile([P, 1], fp32, name="mn", tag="mn") for _ in range(CT)]
            ss = [stats.tile([P, 1], fp32, name="ss", tag="ss") for _ in range(CT)]
            rg = [stats.tile([P, 1], fp32, name="rg", tag="rg") for _ in range(CT)]
            etmps = {}

            # PE: tails + mirrors (attT0 + out emitted right after, below)
            # ACT: mirror-source copies ordered so PE never waits long
            def tail(i):
                for kk in range(lkb):
                    energy_k(lk0 + kk, i, stop=(kk == lkb - 1))

            def etmp_copy(i, j):
                t = stats.tile([P, P], fp32, name="etmp", tag="etmp")
                etmps[(i, j)] = t
                nc.scalar.copy(out=t, in_=Es[j][:, i * P : (i + 1) * P])

            def mirror(i, j):
                nc.tensor.transpose(
                    Es[i][:, j * P : (j + 1) * P], etmps[(i, j)], ident32
                )

            def rowmin(i):
                nc.vector.tensor_reduce(out=mn[i], in_=Es[i], axis=AX, op=ALU.min)

            def exp(i):
                nc.scalar.activation(
                    out=att[:, i, :],
                    in_=Es[i],
                    func=ACT_EXP,
                    bias=mn[i],
                    scale=-1.0,
                    accum_out=ss[i],
                )

            def rgi(i):
                nc.vector.reciprocal(out=rg[i], in_=ss[i])
                nc.vector.tensor_mul(rg[i], rg[i], gam)

            def attT_pe(i):
                tpA = ps.tile([P, 2 * FD], bf16, name="tpA", tag="ps")
                for j in range(CT):
                    nc.tensor.transpose(
                        tpA[:, j * P : (j + 1) * P],
                        att[:, i, j * P : (j + 1) * P],
                        ident,
                    )
                nc.scalar.copy(
                    out=attT[:, i, :, :],
                    in_=tpA[:, 0:C].rearrange("p (j v) -> p j v", v=P),
                )

            # interleaved emission; Tile turns cross-engine deps into sems
            tail(0)
            rowmin(0)
            etmp_copy(1, 0)
            etmp_copy(2, 0)
            etmp_copy(3, 0)
            tail(1)
            mirror(1, 0)
            rowmin(1)
            etmp_copy(2, 1)
            etmp_copy(3, 1)
            exp(0)
            tail(2)
            mirror(2, 0)
            mirror(2, 1)
            rowmin(2)
            rgi(0)
            etmp_copy(3, 2)
            attT_pe(0)
            tail(3)
            mirror(3, 0)
            mirror(3, 1)
            mirror(3, 2)
            rowmin(3)
            exp(1)
            rgi(1)

            # ---- out phase: per i, per 512-col chunk ----
            def out_chunk(i, nh, ot):
                hsl = slice(nh * FD, (nh + 1) * FD)
                ops = ps.tile([P, FD], fp32, name="ops", tag="ps")
                for j in range(CT):
                    nc.tensor.matmul(
                        ops,
                        lhsT=attT[:, i, j, :],
                        rhs=q_bf[:, j, hsl],
                        start=(j == 0),
                        stop=(j == CT - 1),
                    )
                nc.vector.scalar_tensor_tensor(
                    out=ot[:, hsl],
                    in0=ops,
                    scalar=rg[i],
                    in1=q[:, i, hsl],
                    op0=ALU.mult,
                    op1=ALU.add,
                )
                st = nc.sync if (i * 8 + nh) % 2 == 0 else nc.scalar
                st.dma_start(out=o_d[i * P : (i + 1) * P, hsl], in_=ot[:, hsl])

            for i in range(CT):
                ot = outp.tile([P, N], fp32, name="ot", tag="ot", bufs=2)
                for nh in range(8):
                    out_chunk(i, nh, ot)
                    # softmax/attT for later rows ride between out chunks
                    if i == 0 and nh == 1:
                        attT_pe(1)
                    if i == 0 and nh == 3:
                        exp(2)
                        rgi(2)
                    if i == 0 and nh == 5:
                        attT_pe(2)
                    if i == 1 and nh == 1:
                        exp(3)
                        rgi(3)
                    if i == 1 and nh == 3:
                        attT_pe(3)

    nc.compile()
    return nc


def _get_nc():
    if "nc" not in _CACHE:
        _CACHE["nc"] = _build_bass()
    return _CACHE["nc"]


def run(x, gamma, **run_kwargs):
    """Run on 8 cores; returns (results_list, BassKernelResults)."""
    from concourse.bass_utils import run_bass_kernel_spmd

    nc = _get_nc()
    x = np.ascontiguousarray(x, dtype=np.float32)
    gamma = np.ascontiguousarray(gamma, dtype=np.float32)
    in_maps = [
        {"x": np.ascontiguousarray(x[b].reshape(C, N)), "gamma": gamma}
        for b in range(B)
    ]
    res = run_bass_kernel_spmd(nc, in_maps, core_ids=list(range(B)), **run_kwargs)
    out = np.stack([r["out"] for r in res.results]).reshape(B, C, H, W)
    return out, res


def kernel(x, gamma):
    out, _ = run(x, gamma)
    return out.astype(np.float32)


# revision 5
# speedup vs baseline: 1.1361x; 1.1361x over previous
"""CAM (channel attention) module kernel for Trainium2, 8-core data-parallel.

Computes, per batch b (one batch per NeuronCore):
    q = x[b].reshape(C, N)                  # C=512, N=4096
    E = q @ q.T                             # [C, C], symmetric
    att = softmax(rowmax(E) - E, axis=-1)   # == exp(rowmin(E)-E)/rowsum
    out = gamma * (att @ q) + x[b]

v4 design (trace-driven; all matmuls bf16, fp32 E in PSUM, exact fp32 +x
via scalar_tensor_tensor so gamma=0 gives out == x exactly):
  - loads: 10 ramped slabs of x viewed as (c p) n -> p c n. Six on the
    sync ring, four on the scalar ring (a HWDGE ring blocks its engine's
    queue once Tile's 4 DMA sems per ring run out, and the scalar queue
    is the ACT engine - so ACT compute must never sit behind a blocked
    load issue).
  - PE warmup: junk matmuls on a DVE-memset scratch right after the
    preamble so the HAM clock gate is released before real work.
  - ALL q transposes on PE via identity into PSUM staging (c-pair tiles),
    gathered to qT by one DVE + one ACT copy per slab. No DMA-xbar: in
    v2 the xbar transposes crawled behind the load stream (+14us).
  - casts fp32->bf16 on DVE only (single producer per tensor avoids
    coarse cross-engine false deps seen in the v3 trace).
  - energy k-outer i-inner per slab (j>=i blocks only); last slab (128
    cols) runs i-outer with stop so E[0] completes ~1us after load end;
    j<i blocks mirrored from E[j] (ACT copy out + PE fp32 transpose).
  - softmax: rowmin (DVE) -> exp(mn-E) on ACT (bf16 att + accum_out row
    sum); rg = gamma/s per partition; attT via PE transposes + ACT copy.
  - out: per 512-col chunk: 4 bf16 matmuls into a PSUM tile that reuses
    the E bank freed by that row's softmax (tag-aliased into the eps
    pool), then DVE scalar_tensor_tensor out = psum*rg + x (exact fp32
    x), store per chunk alternating sync/gpsimd rings.
"""

import sys

import numpy as np

for _p in ("/opt/trn_rl_repo",):
    if _p not in sys.path:
        sys.path.insert(0, _p)

B, C, H, W = 8, 512, 64, 64
N = H * W  # 4096
P = 128
CT = C // P  # 4 channel tiles
KT = N // P  # 32 spatial tiles
FD = 512  # matmul free-dim / PSUM bank width (fp32)

# ramped load slabs (columns); sum = 4096; last slab small = short tail
SLAB_COLS = [256, 256, 512, 512, 512, 512, 512, 512, 384, 128]
SCALAR_SLABS = {1, 3, 5, 7}  # <= 4 loads on the scalar ring (sem budget)
assert sum(SLAB_COLS) == N

_CACHE = {}


def _build_bass():
    import concourse.mybir as mybir
    import concourse.tile as tile
    from concourse import bacc
    from concourse.masks import make_identity

    fp32 = mybir.dt.float32
    bf16 = mybir.dt.bfloat16
    AX = mybir.AxisListType.X
    ALU = mybir.AluOpType
    ACT_EXP = mybir.ActivationFunctionType.Exp

    nc = bacc.Bacc(None, target_bir_lowering=False, debug=False)
    x_d = nc.dram_tensor("x", [C, N], fp32, kind="ExternalInput")
    g_d = nc.dram_tensor("gamma", [1], fp32, kind="ExternalInput")
    o_d = nc.dram_tensor("out", [C, N], fp32, kind="ExternalOutput")

    with tile.TileContext(nc) as tc:
        with (
            tc.tile_pool(name="persist", bufs=1) as persist,
            tc.tile_pool(name="stats", bufs=4) as stats,
            tc.tile_pool(name="outp", bufs=4) as outp,
            tc.tile_pool(name="eps", bufs=1, space="PSUM") as eps,
            tc.tile_pool(name="tps", bufs=4, space="PSUM") as tps,
        ):
            gam = persist.tile([P, 1], fp32)
            ident = persist.tile([P, P], bf16)
            ident32 = persist.tile([P, P], fp32)
            scratch = persist.tile([P, P], bf16)
            q = persist.tile([P, CT, N], fp32)
            q_bf = persist.tile([P, CT, N], bf16)
            # qT[p, k, c, v] = q[c*128+v, k*128+p]; energy rhs for block k
            # is the contiguous [128, 512] slab qT[:, k, :, :]
            qT = persist.tile([P, KT, CT, P], bf16)
            att = persist.tile([P, CT, C], bf16)
            attT = persist.tile([P, CT, CT, P], bf16)

            # ---- slab loads, issued first so the rings drain FIFO ----
            xv = x_d[:, :].rearrange("(c p) n -> p c n", p=P)
            slabs = []
            c0 = 0
            for s, w in enumerate(SLAB_COLS):
                sl = slice(c0, c0 + w)
                slabs.append((sl, c0 // P, w // P))  # (cols, k0, kb)
                c0 += w
                ring = nc.scalar if s in SCALAR_SLABS else nc.sync
                ring.dma_start(out=q[:, :, sl], in_=xv[:, :, sl])

            # ---- PE warmup: junk matmuls to release the HAM clock gate ----
            nc.vector.memset(scratch, 0.0)
            for _ in range(30):
                wp = tps.tile([P, FD], fp32, name="wp", tag="tps")
                nc.tensor.matmul(
                    wp[:, 0:P], lhsT=scratch, rhs=scratch, start=True, stop=True
                )
            make_identity(nc, ident)
            make_identity(nc, ident32)
            nc.gpsimd.dma_start(out=gam, in_=g_d[:].to_broadcast((P, 1)))

            Es = [
                eps.tile([P, C], fp32, name=f"E{i}", tag=f"E{i}", bufs=1)
                for i in range(CT)
            ]

            def cast(sl, cp):
                nc.vector.tensor_copy(
                    out=q_bf[:, 2 * cp : 2 * cp + 2, sl],
                    in_=q[:, 2 * cp : 2 * cp + 2, sl],
                )

            def transpose_gather(sl, k0, kb, cp):
                # tp holds [k, c(2), v] interleaved so one copy lands both c's
                tp = tps.tile([P, 2 * FD], bf16, name="tp", tag="tps")
                for kk in range(kb):
                    for cc in range(2):
                        nc.tensor.transpose(
                            tp[:, (kk * 2 + cc) * P : (kk * 2 + cc + 1) * P],
                            q_bf[
                                :,
                                2 * cp + cc,
                                sl.start + kk * P : sl.start + (kk + 1) * P,
                            ],
                            ident,
                        )
                src = tp[:, 0 : kb * 2 * P].rearrange(
                    "p (k c v) -> p k c v", c=2, v=P
                )
                dst = qT[:, k0 : k0 + kb, 2 * cp : 2 * cp + 2, :]
                if cp == 0:
                    nc.vector.tensor_copy(out=dst, in_=src)
                else:
                    nc.scalar.copy(out=dst, in_=src)

            def energy_k(k, i, stop=False):
                nc.tensor.matmul(
                    Es[i][:, i * P :],
                    lhsT=qT[:, k, i, :],
                    rhs=qT[:, k, i:, :],
                    start=(k == 0),
                    stop=stop,
                )

            # ---- slab pipeline (all but last slab: k-outer, i-inner) ----
            for s, (sl, k0, kb) in enumerate(slabs):
                for cp in range(2):
                    cast(sl, cp)
                for cp in range(2):
                    transpose_gather(sl, k0, kb, cp)
                if s < len(slabs) - 1:
                    for kk in range(kb):
                        for i in range(CT):
                            energy_k(k0 + kk, i)

            # ---- transition: last-slab energy i-outer + mirrors/softmax ----
            lsl, lk0, lkb = slabs[-1]
            mn = [stats.tile([P, 1], fp32, name="mn", tag="mn") for _ in range(CT)]
            ss = [stats.tile([P, 1], fp32, name="ss", tag="ss") for _ in range(CT)]
            rg = [stats.tile([P, 1], fp32, name="rg", tag="rg") for _ in range(CT)]
            etmps = {}

            def tail(i):
                for kk in range(lkb):
                    energy_k(lk0 + kk, i, stop=(kk == lkb - 1))

            def etmp_copy(i, j):
                t = stats.tile([P, P], fp32, name="etmp", tag="etmp")
                etmps[(i, j)] = t
                nc.scalar.copy(out=t, in_=Es[j][:, i * P : (i + 1) * P])

            def mirror(i, j):
                nc.tensor.transpose(
                    Es[i][:, j * P : (j + 1) * P], etmps[(i, j)], ident32
                )

            def rowmin(i):
                nc.vector.tensor_reduce(out=mn[i], in_=Es[i], axis=AX, op=ALU.min)

            def exp(i):
                nc.scalar.activation(
                    out=att[:, i, :],
                    in_=Es[i],
                    func=ACT_EXP,
                    bias=mn[i],
                    scale=-1.0,
                    accum_out=ss[i],
                )

            def rgi(i):
                nc.vector.reciprocal(out=rg[i], in_=ss[i])
                nc.vector.tensor_mul(rg[i], rg[i], gam)

            def attT_pe(i):
                tpA = tps.tile([P, 2 * FD], bf16, name="tpA", tag="tps")
                for j in range(CT):
                    nc.tensor.transpose(
                        tpA[:, j * P : (j + 1) * P],
                        att[:, i, j * P : (j + 1) * P],
                        ident,
                    )
                nc.scalar.copy(
                    out=attT[:, i, :, :],
                    in_=tpA[:, 0:C].rearrange("p (j v) -> p j v", v=P),
                )

            # out accumulators alias the E banks freed by each row's softmax
            def out_chunk(i, nh, ot):
                hsl = slice(nh * FD, (nh + 1) * FD)
                ops = eps.tile([P, FD], fp32, name="ops", tag=f"E{nh % CT}", bufs=1)
                for j in range(CT):
                    nc.tensor.matmul(
                        ops,
                        lhsT=attT[:, i, j, :],
                        rhs=q_bf[:, j, hsl],
                        start=(j == 0),
                        stop=(j == CT - 1),
                    )
                nc.vector.scalar_tensor_tensor(
                    out=ot[:, hsl],
                    in0=ops,
                    scalar=rg[i],
                    in1=q[:, i, hsl],
                    op0=ALU.mult,
                    op1=ALU.add,
                )
                st = nc.sync if (i * 8 + nh) % 2 == 1 else nc.gpsimd
                st.dma_start(out=o_d[i * P : (i + 1) * P, hsl], in_=ot[:, hsl])

            ots = [
                outp.tile([P, N], fp32, name="ot", tag="ot", bufs=2)
                for _ in range(CT)
            ]

            # interleaved emission; per-engine program order is preserved and
            # Tile turns cross-engine deps into sems. E-bank reuse rule: all
            # readers of E{b} must be emitted before out_chunk nh%4==b of i=0.
            tail(0)
            rowmin(0)
            etmp_copy(1, 0)
            etmp_copy(2, 0)
            etmp_copy(3, 0)
            tail(1)
            tail(2)
            tail(3)
            exp(0)
            mirror(1, 0)
            attT_pe(0)
            rowmin(1)
            rgi(0)
            out_chunk(0, 0, ots[0])
            etmp_copy(2, 1)
            etmp_copy(3, 1)
            mirror(2, 0)
            mirror(2, 1)
            rowmin(2)
            exp(1)
            rgi(1)
            out_chunk(0, 1, ots[0])
            etmp_copy(3, 2)
            mirror(3, 0)
            mirror(3, 1)
            mirror(3, 2)
            rowmin(3)
            exp(2)
            rgi(2)
            out_chunk(0, 2, ots[0])
            attT_pe(1)
            exp(3)
            rgi(3)
            out_chunk(0, 3, ots[0])
            for nh in range(4, 8):
                out_chunk(0, nh, ots[0])
                if nh == 5:
                    attT_pe(2)
            for i in range(1, CT):
                for nh in range(8):
                    out_chunk(i, nh, ots[i])
                    if i == 1 and nh == 3:
                        attT_pe(3)

    nc.compile()
    return nc


def _get_nc():
    if "nc" not in _CACHE:
        _CACHE["nc"] = _build_bass()
    return _CACHE["nc"]


def run(x, gamma, **run_kwargs):
    """Run on 8 cores; returns (results_list, BassKernelResults)."""
    from concourse.bass_utils import run_bass_kernel_spmd

    nc = _get_nc()
    x = np.ascontiguousarray(x, dtype=np.float32)
    gamma = np.ascontiguousarray(gamma, dtype=np.float32)
    in_maps = [
        {"x": np.ascontiguousarray(x[b].reshape(C, N)), "gamma": gamma}
        for b in range(B)
    ]
    res = run_bass_kernel_spmd(nc, in_maps, core_ids=list(range(B)), **run_kwargs)
    out = np.stack([r["out"] for r in res.results]).reshape(B, C, H, W)
    return out, res


def kernel(x, gamma):
    out, _ = run(x, gamma)
    return out.astype(np.float32)


# revision 9
# speedup vs baseline: 1.2969x; 1.1416x over previous
"""CAM (channel attention) module kernel for Trainium2, 8-core data-parallel.

Computes, per batch b (one batch per NeuronCore):
    q = x[b].reshape(C, N)                  # C=512, N=4096
    E = q @ q.T                             # [C, C], symmetric
    att = softmax(rowmax(E) - E, axis=-1)   # == exp(rowmin(E)-E)/rowsum
    out = gamma * (att @ q) + x[b]

v4 design (trace-driven; all matmuls bf16, fp32 E in PSUM, exact fp32 +x
via scalar_tensor_tensor so gamma=0 gives out == x exactly):
  - loads: 10 ramped slabs of x viewed as (c p) n -> p c n. Six on the
    sync ring, four on the scalar ring (a HWDGE ring blocks its engine's
    queue once Tile's 4 DMA sems per ring run out, and the scalar queue
    is the ACT engine - so ACT compute must never sit behind a blocked
    load issue).
  - PE warmup: junk matmuls on a DVE-memset scratch right after the
    preamble so the HAM clock gate is released before real work.
  - ALL q transposes on PE via identity into PSUM staging (c-pair tiles),
    gathered to qT by one DVE + one ACT copy per slab. No DMA-xbar: in
    v2 the xbar transposes crawled behind the load stream (+14us).
  - casts fp32->bf16 on DVE only (single producer per tensor avoids
    coarse cross-engine false deps seen in the v3 trace).
  - energy k-outer i-inner per slab (j>=i blocks only); last slab (128
    cols) runs i-outer with stop so E[0] completes ~1us after load end;
    j<i blocks mirrored from E[j] (ACT copy out + PE fp32 transpose).
  - softmax: rowmin (DVE) -> exp(mn-E) on ACT (bf16 att + accum_out row
    sum); rg = gamma/s per partition; attT via PE transposes + ACT copy.
  - out: per 512-col chunk: 4 bf16 matmuls into a PSUM tile that reuses
    the E bank freed by that row's softmax (tag-aliased into the eps
    pool), then DVE scalar_tensor_tensor out = psum*rg + x (exact fp32
    x), store per chunk alternating sync/gpsimd rings.
"""

import sys

import numpy as np

for _p in ("/opt/trn_rl_repo",):
    if _p not in sys.path:
        sys.path.insert(0, _p)

B, C, H, W = 8, 512, 64, 64
N = H * W  # 4096
P = 128
CT = C // P  # 4 channel tiles
KT = N // P  # 32 spatial tiles
FD = 512  # matmul free-dim / PSUM bank width (fp32)

# ramped load slabs (columns); sum = 4096; last slab small = short tail
SLAB_COLS = [256, 256, 512, 512, 512, 512, 512, 512, 384, 128]
SCALAR_SLABS = {1, 3, 5, 7, 9}  # alternate rings: each HWDGE queue caps ~195GB/s
assert sum(SLAB_COLS) == N

_CACHE = {}


def _build_bass():
    import concourse.mybir as mybir
    import concourse.tile as tile
    from concourse import bacc
    from concourse.masks import make_identity

    fp32 = mybir.dt.float32
    bf16 = mybir.dt.bfloat16
    AX = mybir.AxisListType.X
    ALU = mybir.AluOpType
    ACT_EXP = mybir.ActivationFunctionType.Exp

    nc = bacc.Bacc(None, target_bir_lowering=False, debug=False)
    x_d = nc.dram_tensor("x", [C, N], fp32, kind="ExternalInput")
    g_d = nc.dram_tensor("gamma", [1], fp32, kind="ExternalInput")
    o_d = nc.dram_tensor("out", [C, N], fp32, kind="ExternalOutput")

    with tile.TileContext(nc) as tc:
        with (
            tc.tile_pool(name="persist", bufs=1) as persist,
            tc.tile_pool(name="stats", bufs=4) as stats,
            tc.tile_pool(name="outp", bufs=4) as outp,
            tc.tile_pool(name="eps", bufs=1, space="PSUM") as eps,
            tc.tile_pool(name="tps", bufs=2, space="PSUM") as tps,
        ):
            gam = persist.tile([P, 1], fp32)
            ident = persist.tile([P, P], bf16)
            ident32 = persist.tile([P, P], fp32)
            scratch = persist.tile([P, P], bf16)
            q = persist.tile([P, CT, N], fp32)
            q_bf = persist.tile([P, CT, N], bf16)
            # qT[p, k, c, v] = q[c*128+v, k*128+p]; energy rhs for block k
            # is the contiguous [128, 512] slab qT[:, k, :, :]
            qT = persist.tile([P, KT, CT, P], bf16)
            att = persist.tile([P, CT, C], bf16)
            attT = persist.tile([P, CT, CT, P], bf16)

            # ---- slab loads, issued first so the rings drain FIFO ----
            xv = x_d[:, :].rearrange("(c p) n -> p c n", p=P)
            slabs = []
            c0 = 0
            for s, w in enumerate(SLAB_COLS):
                sl = slice(c0, c0 + w)
                slabs.append((sl, c0 // P, w // P))  # (cols, k0, kb)
                c0 += w
                ring = nc.scalar if s in SCALAR_SLABS else nc.sync
                ring.dma_start(out=q[:, :, sl], in_=xv[:, :, sl])

            Es = [
                eps.tile([P, C], fp32, name=f"E{i}", tag=f"E{i}", bufs=1)
                for i in range(CT)
            ]

            # ---- PE warmup: junk matmuls to release the HAM clock gate.
            # Target E0's bank: energy k=0 start=True resets it anyway.
            nc.vector.memset(scratch, 0.0)
            for _ in range(25):
                nc.tensor.matmul(
                    Es[0][:, 0:P], lhsT=scratch, rhs=scratch, start=True, stop=True
                )
            make_identity(nc, ident)
            make_identity(nc, ident32)
            nc.gpsimd.dma_start(out=gam, in_=g_d[:].to_broadcast((P, 1)))

            def cast(sl, c):
                nc.vector.tensor_copy(out=q_bf[:, c, sl], in_=q[:, c, sl])

            def transpose_gather(sl, k0, kb, s):
                # tp holds [k, c(4), v] interleaved: ONE contiguous gather
                # lands the whole slab (strided writes get split by bass
                # into per-run ops with ~150ns overhead each - avoid).
                tp = tps.tile([P, 4 * FD], bf16, name="tp", tag="tps")
                for kk in range(kb):
                    for c in range(CT):
                        nc.tensor.transpose(
                            tp[:, (kk * 4 + c) * P : (kk * 4 + c + 1) * P],
                            q_bf[
                                :, c, sl.start + kk * P : sl.start + (kk + 1) * P
                            ],
                            ident,
                        )
                src = tp[:, 0 : kb * 4 * P].rearrange(
                    "p (k c v) -> p k c v", c=CT, v=P
                )
                dst = qT[:, k0 : k0 + kb, :, :]
                # early gathers on DVE; from s4 the scalar ring's load-issue
                # queue has drained, so ACT is safe to use
                if s < 4:
                    nc.vector.tensor_copy(out=dst, in_=src)
                else:
                    nc.scalar.copy(out=dst, in_=src)

            def energy_k(k, i, stop=False):
                nc.tensor.matmul(
                    Es[i][:, i * P :],
                    lhsT=qT[:, k, i, :],
                    rhs=qT[:, k, i:, :],
                    start=(k == 0),
                    stop=stop,
                )

            # ---- slab pipeline (all but last slab: k-outer, i-inner) ----
            for s, (sl, k0, kb) in enumerate(slabs):
                for c in range(CT):
                    cast(sl, c)
                transpose_gather(sl, k0, kb, s)
                if s < len(slabs) - 1:
                    for kk in range(kb):
                        for i in range(CT):
                            energy_k(k0 + kk, i)

            # ---- transition: last-slab energy i-outer + mirrors/softmax ----
            lsl, lk0, lkb = slabs[-1]
            mn = [stats.tile([P, 1], fp32, name="mn", tag="mn") for _ in range(CT)]
            ss = [stats.tile([P, 1], fp32, name="ss", tag="ss") for _ in range(CT)]
            rg = [stats.tile([P, 1], fp32, name="rg", tag="rg") for _ in range(CT)]
            etmps = {}

            def tail(i):
                for kk in range(lkb):
                    energy_k(lk0 + kk, i, stop=(kk == lkb - 1))

            def etmp_copy(i, j):
                t = stats.tile([P, P], fp32, name="etmp", tag="etmp")
                etmps[(i, j)] = t
                nc.scalar.copy(out=t, in_=Es[j][:, i * P : (i + 1) * P])

            def mirror(i, j):
                nc.tensor.transpose(
                    Es[i][:, j * P : (j + 1) * P], etmps[(i, j)], ident32
                )

            def rowmin(i):
                nc.vector.tensor_reduce(out=mn[i], in_=Es[i], axis=AX, op=ALU.min)

            def exp(i):
                nc.scalar.activation(
                    out=att[:, i, :],
                    in_=Es[i],
                    func=ACT_EXP,
                    bias=mn[i],
                    scale=-1.0,
                    accum_out=ss[i],
                )

            def rgi(i):
                nc.vector.reciprocal(out=rg[i], in_=ss[i])
                nc.vector.tensor_mul(rg[i], rg[i], gam)

            def attT_pe(i):
                tpA = tps.tile([P, 2 * FD], bf16, name="tpA", tag="tps")
                for j in range(CT):
                    nc.tensor.transpose(
                        tpA[:, j * P : (j + 1) * P],
                        att[:, i, j * P : (j + 1) * P],
                        ident,
                    )
                nc.scalar.copy(
                    out=attT[:, i, :, :],
                    in_=tpA[:, 0:C].rearrange("p (j v) -> p j v", v=P),
                )

            # out accumulators alias the E banks freed by each row's softmax
            def out_chunk(i, nh, ot):
                hsl = slice(nh * FD, (nh + 1) * FD)
                ops = eps.tile([P, FD], fp32, name="ops", tag=f"E{nh % CT}", bufs=1)
                for j in range(CT):
                    nc.tensor.matmul(
                        ops,
                        lhsT=attT[:, i, j, :],
                        rhs=q_bf[:, j, hsl],
                        start=(j == 0),
                        stop=(j == CT - 1),
                    )
                nc.vector.scalar_tensor_tensor(
                    out=ot[:, hsl],
                    in0=ops,
                    scalar=rg[i],
                    in1=q[:, i, hsl],
                    op0=ALU.mult,
                    op1=ALU.add,
                )
                st = nc.sync if (i * 8 + nh) % 2 == 1 else nc.gpsimd
                st.dma_start(out=o_d[i * P : (i + 1) * P, hsl], in_=ot[:, hsl])

            ots = [
                outp.tile([P, N], fp32, name="ot", tag="ot", bufs=2)
                for _ in range(CT)
            ]

            # interleaved emission; per-engine program order is preserved and
            # Tile turns cross-engine deps into sems. E-bank reuse rule: all
            # readers of E{b} must be emitted before out_chunk nh%4==b of i=0.
            tail(0)
            rowmin(0)
            etmp_copy(1, 0)
            etmp_copy(2, 0)
            etmp_copy(3, 0)
            tail(1)
            tail(2)
            tail(3)
            exp(0)
            mirror(1, 0)
            attT_pe(0)
            rowmin(1)
            rgi(0)
            out_chunk(0, 0, ots[0])
            etmp_copy(2, 1)
            etmp_copy(3, 1)
            mirror(2, 0)
            mirror(2, 1)
            rowmin(2)
            exp(1)
            rgi(1)
            out_chunk(0, 1, ots[0])
            etmp_copy(3, 2)
            mirror(3, 0)
            mirror(3, 1)
            mirror(3, 2)
            rowmin(3)
            exp(2)
            rgi(2)
            out_chunk(0, 2, ots[0])
            attT_pe(1)
            exp(3)
            rgi(3)
            out_chunk(0, 3, ots[0])
            for nh in range(4, 8):
                out_chunk(0, nh, ots[0])
                if nh == 5:
                    attT_pe(2)
            for i in range(1, CT):
                for nh in range(8):
                    out_chunk(i, nh, ots[i])
                    if i == 1 and nh == 3:
                        attT_pe(3)

    nc.compile()
    return nc


def _get_nc():
    if "nc" not in _CACHE:
        _CACHE["nc"] = _build_bass()
    return _CACHE["nc"]


def run(x, gamma, **run_kwargs):
    """Run on 8 cores; returns (results_list, BassKernelResults)."""
    from concourse.bass_utils import run_bass_kernel_spmd

    nc = _get_nc()
    x = np.ascontiguousarray(x, dtype=np.float32)
    gamma = np.ascontiguousarray(gamma, dtype=np.float32)
    in_maps = [
        {"x": np.ascontiguousarray(x[b].reshape(C, N)), "gamma": gamma}
        for b in range(B)
    ]
    res = run_bass_kernel_spmd(nc, in_maps, core_ids=list(range(B)), **run_kwargs)
    out = np.stack([r["out"] for r in res.results]).reshape(B, C, H, W)
    return out, res


def kernel(x, gamma):
    out, _ = run(x, gamma)
    return out.astype(np.float32)
